# revision 1
# baseline (speedup 1.0000x reference)
"""Trainium2 Bass kernel for nn_Encoder_31550829756513 (2-layer dual-branch GCN).

Strategy (8 NeuronCores, node-partitioned graph parallel):
  Host:
    - Factorize both branches' sym-norm edge weights into
        src-side  dis_g[src]  (folded into the table rows / x pre-scale)
        per-edge  s_e = dis_p[src]*ppmi_e*dis_p[dst] / dis_g[src]
        dst-side  dis_g[dst]  (per-block post-scale on the Act engine)
      so the GCN branch needs no per-edge arithmetic (identity-matmul PSUM
      accumulate) and the PPMI branch needs one tensor_scalar per chunk.
      Self loops are just edges with s_e = dis_p^2/dis_g.
    - Nodes are ranked by in-degree and dealt to (slot s = r//1024, core c,
      partition p).  A chunk k of block (c,s) holds the k-th in-edge of each
      of the 128 dst nodes, so chunk slots map 1:1 onto PSUM rows.
    - Gathers use dma_gather (int16 row indices) over 4 windows of 25088
      table rows (= 2 cores each).  A greedy pass picks each source node's
      core so that every destination's in-edges spread evenly over the 4
      windows (keeps the per-window chunk count near deg/4).  Padding slots
      index the window's last row, which is reserved for a pad node (zeros).
  Device (single SPMD program; per-core behaviour comes only from inputs):
    - Phase A: every core computes the FULL table t[n] = dis_g[n]*(x'@W1) in
      fp16 to local DRAM (cheaper than an AllGather of the same table).
    - L1: per block: 4 window dma_gathers; per chunk one DVE tensor_scalar
      and two identity-matmul PSUM accumulations; epilogue computes
      h1 = relu(post-scale), DMA-transposes, and projects y' =
      dis_g[n]*[h1_g@W2 | h1_p@W2] (128 wide).
    - AllGather of y' (the only collective).
    - L2: same message pass over [y'_g|y'_p]; logits l = z.dense_w on DVE,
      wg = sigmoid(lg-lp), softmax blend.
  Host: un-permute rows, slice to N.
"""

import os
import numpy as np

P = 128
_FP16 = np.float16


class Cfg:
    def __init__(self, n, e, d=256, h=128, o=64, ncores=8):
        self.N = n
        self.E = e
        self.D = d
        self.H = h
        self.O = o
        self.W = 128                       # table/y row width (elements)
        self.ncores = ncores
        self.NB = 98                       # slots (blocks) per core
        self.CORE_ROWS = self.NB * P       # 12544
        self.NPAD = ncores * self.CORE_ROWS  # 100352
        self.NBLK_PAD = self.NPAD // P     # 784
        self.GL = 7                        # blocks per group (per-core loops)
        self.NGRP = self.NB // self.GL     # 14
        self.NW = 4                        # gather windows
        self.WROWS = self.NPAD // self.NW  # 25088 rows per window


FULL = Cfg(100000, 1600000)


# ----------------------------------------------------------------------------
# Host preprocessing
# ----------------------------------------------------------------------------

def _greedy_windows(cfg, slot_of_rank, order_ext, rows_all, cols_all, n):
    """Per slot, assign its 1024 nodes to 4 windows (256 each), greedily
    balancing every destination's window in-degree.  Returns (core, part)
    per rank."""
    npad = cfg.NPAD
    # CSR over sources (edges incl self loops)
    so = np.argsort(rows_all, kind="stable")
    sc = np.bincount(rows_all, minlength=npad)
    sstart = np.concatenate([[0], np.cumsum(sc)[:-1]])
    dst_sorted = cols_all[so]

    outdeg = sc
    cnt = np.zeros((cfg.NW, n), dtype=np.int32)
    core_of = np.empty(npad, dtype=np.int64)
    part_of = np.empty(npad, dtype=np.int64)

    slot_width = cfg.ncores * P
    for s in range(cfg.NB):
        ranks = np.arange(s * slot_width, (s + 1) * slot_width)
        nodes = order_ext[ranks]                      # orig id or >= n (pad)
        is_pad = nodes >= n
        od = np.where(is_pad, -1, outdeg[np.minimum(nodes, n - 1)])
        proc = np.argsort(-od, kind="stable")         # real, busy first
        quota = np.full(cfg.NW, 2 * P, dtype=np.int64)
        wlists = [[] for _ in range(cfg.NW)]
        for li in proc:
            node = nodes[li]
            if node >= n or outdeg[node] == 0:
                w = int(np.argmax(quota))
            else:
                d = dst_sorted[sstart[node]:sstart[node] + sc[node]]
                m = cnt[:, d]
                # minimize the resulting per-dst max, then the squared sum
                scores = (m.max(axis=1) * 1e6
                          + (m.astype(np.float64) ** 2).sum(axis=1))
                scores[quota == 0] = np.inf
                w = int(np.argmin(scores))
                np.add.at(cnt[w], d, 1)
            quota[w] -= 1
            wlists[w].append(ranks[li])
        for w in range(cfg.NW):
            lst = np.array(wlists[w])
            rl = order_ext[lst]
            pads_last = np.argsort((rl >= n).astype(np.int8), kind="stable")
            lst = lst[pads_last]
            core_of[lst[:P]] = 2 * w
            part_of[lst[:P]] = np.arange(P)
            core_of[lst[P:]] = 2 * w + 1
            part_of[lst[P:]] = np.arange(P)
    return core_of, part_of


def _preprocess(cfg, x, edge_index, ppmi_edge_weight, W1, b1, W2, b2,
                dense_w, dense_b):
    n, e = cfg.N, cfg.E
    row = np.asarray(edge_index[0], dtype=np.int64)
    col = np.asarray(edge_index[1], dtype=np.int64)
    ppmi = np.asarray(ppmi_edge_weight, dtype=np.float64)

    sl = np.arange(n, dtype=np.int64)
    row_sl = np.concatenate([row, sl])
    deg_g = np.bincount(row_sl, weights=np.concatenate(
        [np.ones(e), np.ones(n)]), minlength=n)
    deg_p = np.bincount(row_sl, weights=np.concatenate(
        [ppmi, np.ones(n)]), minlength=n)
    dis_g = np.where(deg_g > 0, deg_g ** -0.5, 0.0)
    dis_p = np.where(deg_p > 0, deg_p ** -0.5, 0.0)

    # edges including self loops; per-edge PPMI-branch scalar
    rows_all = row_sl
    cols_all = np.concatenate([col, sl])
    sval_all = np.concatenate([
        dis_p[row] * ppmi * dis_p[col] / dis_g[row],
        (dis_p ** 2) / dis_g,
    ])

    # ---- degree-sorted rank assignment (by in-degree incl self) -----------
    indeg = np.bincount(cols_all, minlength=n)
    indeg_ext = np.full(cfg.NPAD, -1, dtype=np.int64)
    indeg_ext[:n] = indeg
    order_ext = np.argsort(-indeg_ext, kind="stable")   # rank -> orig (or pad)
    r_of = np.empty(cfg.NPAD, dtype=np.int64)
    r_of[order_ext] = np.arange(cfg.NPAD)

    slot_width = cfg.ncores * P
    slot_of_rank = np.arange(cfg.NPAD) // slot_width

    core_of_rank, part_of_rank = _greedy_windows(
        cfg, slot_of_rank, order_ext, rows_all, cols_all, n)
    window_of_rank = core_of_rank // 2

    # ---- profile-clustered re-deal ----------------------------------------
    # Per-node window-degree profile (as a destination); group nodes with
    # similar profiles into the same slot so the per-slot max is tight.
    wdeg_node = np.zeros((cfg.NPAD, cfg.NW), dtype=np.int64)
    w_src0 = window_of_rank[r_of[rows_all]]
    np.add.at(wdeg_node, (cols_all, w_src0), 1)   # dst indexed by orig id < n
    prof = np.zeros((cfg.NPAD, cfg.NW), dtype=np.int64)
    real0 = order_ext < n
    prof[np.arange(cfg.NPAD)[real0]] = wdeg_node[order_ext[real0]]
    # (profiles are per orig node; map rank -> profile via order_ext)
    psum = prof.sum(axis=1)
    pmax = prof.max(axis=1)
    key_order = np.lexsort((-prof[:, 3], -prof[:, 2], -prof[:, 1],
                            -prof[:, 0], -pmax, -psum))   # over ranks
    # per-window queues in key order
    w_of_rank0 = window_of_rank
    new_rank = np.empty(cfg.NPAD, dtype=np.int64)
    pos = 0
    queues = [key_order[w_of_rank0[key_order] == w] for w in range(cfg.NW)]
    ptr = [0] * cfg.NW
    core_of_rank = np.empty(cfg.NPAD, dtype=np.int64)
    part_of_rank = np.empty(cfg.NPAD, dtype=np.int64)
    slot_new = np.empty(cfg.NPAD, dtype=np.int64)
    for s in range(cfg.NB):
        for w in range(cfg.NW):
            take = queues[w][ptr[w]:ptr[w] + 2 * P]
            ptr[w] += 2 * P
            slot_new[take] = s
            core_of_rank[take[:P]] = 2 * w
            part_of_rank[take[:P]] = np.arange(P)
            core_of_rank[take[P:]] = 2 * w + 1
            part_of_rank[take[P:]] = np.arange(P)
    slot_of_rank = slot_new
    window_of_rank = core_of_rank // 2

    s_a = slot_of_rank
    g_a = s_a // cfg.GL
    t_a = s_a % cfg.GL
    trow_of_rank = (core_of_rank * cfg.CORE_ROWS + g_a * (cfg.GL * P)
                    + part_of_rank * cfg.GL + t_a)
    bm_of_rank = core_of_rank * cfg.CORE_ROWS + s_a * P + part_of_rank
    assert np.array_equal(trow_of_rank // cfg.WROWS, window_of_rank)
    # reserved zero rows: last row of each window must be a pad node
    inv_trow = np.empty(cfg.NPAD, dtype=np.int64)
    inv_trow[trow_of_rank] = np.arange(cfg.NPAD)
    for w in range(cfg.NW):
        rk = inv_trow[(w + 1) * cfg.WROWS - 1]
        assert order_ext[rk] >= n, "window-last row must be a pad node"

    # ---- per-edge placement ----------------------------------------------
    ne = rows_all.shape[0]
    rd = r_of[cols_all]
    rs = r_of[rows_all]
    s_d = slot_of_rank[rd]
    c_d = core_of_rank[rd]
    p_d = part_of_rank[rd]
    w_s = window_of_rank[rs]
    lrow_s = trow_of_rank[rs] - w_s * cfg.WROWS

    # k rank within (dst, window)
    gkey = rd * cfg.NW + w_s
    o2 = np.argsort(gkey, kind="stable")
    gk_sorted = gkey[o2]
    first = np.searchsorted(gk_sorted, gk_sorted)
    k_e = np.empty(ne, dtype=np.int64)
    k_e[o2] = np.arange(ne) - first

    # per (slot, window) chunk counts: max over dsts in the slot
    wdeg = np.bincount(gkey, minlength=cfg.NPAD * cfg.NW)
    wdeg = wdeg.reshape(cfg.NPAD, cfg.NW)
    Kw = np.zeros((cfg.NB, cfg.NW), dtype=np.int64)
    rank_slot = slot_of_rank
    for w in range(cfg.NW):
        Kw[:, w] = np.maximum.reduceat(
            wdeg[:, w][np.argsort(rank_slot, kind="stable")],
            np.arange(cfg.NB) * slot_width)
    assert np.all(k_e < Kw[s_d, w_s])

    C0 = np.zeros((cfg.NB, cfg.NW), dtype=np.int64)
    np.cumsum(Kw.reshape(-1)[:-1], out=C0.reshape(-1)[1:])
    NCHT = int(Kw.sum())

    # int16 idx slab in 16-partition wrap layout, replicated x8
    PAD_LIDX = cfg.WROWS - 1
    idx16 = np.full((cfg.ncores, 16, NCHT * 8), PAD_LIDX, dtype=np.int16)
    sval = np.zeros((cfg.ncores, P, NCHT), dtype=np.float32)

    colpos = C0[s_d, w_s] + k_e                    # chunk column in schedule
    i_pos = k_e * P + p_d                          # position within gather
    i16p = (i_pos % 16).astype(np.int64)
    i16c = (C0[s_d, w_s] * 8 + i_pos // 16).astype(np.int64)
    idx16[c_d, i16p, i16c] = lrow_s.astype(np.int16)
    sval[c_d, p_d, colpos] = sval_all.astype(np.float32)

    disg_slab = np.zeros((cfg.ncores, P, cfg.NB), dtype=np.float32)
    real_rank = order_ext < n
    dval = np.zeros(cfg.NPAD)
    dval[real_rank] = dis_g[order_ext[real_rank]]
    disg_slab[core_of_rank, part_of_rank, s_a] = dval.astype(np.float32)

    # x' = dis_g[n] * x, block-major (bm) columns, transposed
    xf = np.zeros((cfg.NPAD, cfg.D), dtype=np.float32)
    xs = np.asarray(x, dtype=np.float32) * dis_g[:, None].astype(np.float32)
    xf[bm_of_rank[real_rank]] = xs[order_ext[real_rank]]
    xT = np.ascontiguousarray(xf.T.astype(_FP16))

    W1f = np.asarray(W1, dtype=np.float32).astype(_FP16)
    W2f = np.asarray(W2, dtype=np.float32).astype(_FP16)
    dwr = np.tile(np.asarray(dense_w, np.float32).reshape(1, -1), (P, 1))

    has_b1 = not np.allclose(np.asarray(b1), 0.0)
    has_b2 = not np.allclose(np.asarray(b2), 0.0)

    in_maps = []
    for c in range(cfg.ncores):
        i16 = np.zeros((P, NCHT * 8), dtype=np.int16)
        for rep in range(8):
            i16[rep * 16:(rep + 1) * 16] = idx16[c]
        m = {
            "xT": xT, "w1": W1f, "w2": np.ascontiguousarray(W2f),
            "dwr": dwr,
            "idx16": i16,
            "sval": np.ascontiguousarray(sval[c]),
            "disg": np.ascontiguousarray(disg_slab[c]),
        }
        if has_b1:
            m["b1r"] = np.tile(np.asarray(b1, np.float32)[None, :], (P, 1))
        if has_b2:
            m["b2r"] = np.tile(np.asarray(b2, np.float32)[None, :], (P, 1))
        in_maps.append(m)

    key = (tuple(int(k) for k in Kw.reshape(-1)), has_b1, has_b2)
    meta = {"trow_of_rank": trow_of_rank, "order_ext": order_ext}
    return in_maps, key, meta


# ----------------------------------------------------------------------------
# Device program
# ----------------------------------------------------------------------------

def build_program(cfg, key):
    from concourse import bass, mybir, tile, bacc

    Kflat, has_b1, has_b2 = key
    Kw = np.array(Kflat, dtype=np.int64).reshape(cfg.NB, cfg.NW)
    C0 = np.zeros((cfg.NB, cfg.NW), dtype=np.int64)
    np.cumsum(Kw.reshape(-1)[:-1], out=C0.reshape(-1)[1:])
    NCHT = int(Kw.sum())

    dt16 = mybir.dt.float16
    dt32 = mybir.dt.float32
    AOT = mybir.AluOpType
    AFT = mybir.ActivationFunctionType

    NB, H, O, D, W, GL = cfg.NB, cfg.H, cfg.O, cfg.D, cfg.W, cfg.GL

    nc = bacc.Bacc("TRN2", debug=False, enable_asserts=False,
                   num_devices=cfg.ncores)

    xT = nc.dram_tensor("xT", [D, cfg.NPAD], dt16, kind="ExternalInput")
    w1 = nc.dram_tensor("w1", [D, H], dt16, kind="ExternalInput")
    w2 = nc.dram_tensor("w2", [H, O], dt16, kind="ExternalInput")
    dwr = nc.dram_tensor("dwr", [P, O], dt32, kind="ExternalInput")
    idx16 = nc.dram_tensor("idx16", [P, NCHT * 8], mybir.dt.int16,
                           kind="ExternalInput")
    sval = nc.dram_tensor("sval", [P, NCHT], dt32, kind="ExternalInput")
    disg = nc.dram_tensor("disg", [P, NB], dt32, kind="ExternalInput")
    if has_b1:
        b1r = nc.dram_tensor("b1r", [P, H], dt32, kind="ExternalInput")
    if has_b2:
        b2r = nc.dram_tensor("b2r", [P, O], dt32, kind="ExternalInput")

    table = nc.dram_tensor("table", [cfg.NPAD, W], dt16)
    y_shard = nc.dram_tensor("y_shard", [cfg.CORE_ROWS, W], dt16)
    y_full = nc.dram_tensor("y_full", [cfg.NPAD, W], dt16,
                            addr_space="Shared")
    outp = nc.dram_tensor("out", [cfg.CORE_ROWS, O], dt32,
                          kind="ExternalOutput")

    groups_all = [list(range(cfg.ncores))]
    table_flat = table.ap().rearrange("a b -> (a b)")
    yshard_flat = y_shard.ap().rearrange("a b -> (a b)")
    out_flat = outp.ap().rearrange("a b -> (a b)")

    GA = GL
    NGA = cfg.NBLK_PAD // GA      # 112 groups over the whole table

    with tile.TileContext(nc) as tc:
        with tc.tile_pool(name="const", bufs=1) as cpool:
            w1a = cpool.tile([P, H], dt16)
            w1b = cpool.tile([P, H], dt16)
            nc.sync.dma_start(out=w1a[:], in_=w1[0:P, :])
            nc.sync.dma_start(out=w1b[:], in_=w1[P:2 * P, :])
            w2sb = cpool.tile([P, O], dt16)
            nc.sync.dma_start(out=w2sb[:], in_=w2[:, :])
            dw_sb = cpool.tile([P, O], dt32)
            nc.sync.dma_start(out=dw_sb[:], in_=dwr[:, :])
            idx_sb = cpool.tile([P, NCHT * 8], mybir.dt.int16)
            nc.scalar.dma_start(out=idx_sb[:], in_=idx16[:, :])
            sval_sb = cpool.tile([P, NCHT], dt32)
            nc.scalar.dma_start(out=sval_sb[:], in_=sval[:, :])
            disg_sb = cpool.tile([P, NB], dt32)
            nc.sync.dma_start(out=disg_sb[:], in_=disg[:, :])
            if has_b1:
                b1sb = cpool.tile([P, H], dt32)
                nc.sync.dma_start(out=b1sb[:], in_=b1r[:, :])
            if has_b2:
                b2sb = cpool.tile([P, O], dt32)
                nc.sync.dma_start(out=b2sb[:], in_=b2r[:, :])

            # ---------------- phase A: full table t = x'@W1 --------------
            with tc.tile_pool(name="pa_x", bufs=3) as xp, \
                 tc.tile_pool(name="pa_ps", bufs=4, space="PSUM") as pp, \
                 tc.tile_pool(name="pa_g", bufs=3) as gp:
                for gi in range(NGA):
                    base = gi * GA * P
                    xa = xp.tile([P, GA * P], dt16, tag="xa")
                    xb = xp.tile([P, GA * P], dt16, tag="xb")
                    nc.sync.dma_start(out=xa[:], in_=xT[0:P, base:base + GA * P])
                    nc.gpsimd.dma_start(out=xb[:],
                                        in_=xT[P:2 * P, base:base + GA * P])
                    gt = gp.tile([P, GA * W], dt16, tag="gt")
                    for t in range(GA):
                        ps = pp.tile([P, H], dt32, tag="ps")
                        nc.tensor.matmul(out=ps[:],
                                         lhsT=xa[:, t * P:(t + 1) * P],
                                         rhs=w1a[:], start=True, stop=False)
                        nc.tensor.matmul(out=ps[:],
                                         lhsT=xb[:, t * P:(t + 1) * P],
                                         rhs=w1b[:], start=False, stop=True)
                        dst = gt[:, t * W:(t + 1) * W]
                        if t % 2 == 0:
                            nc.vector.tensor_copy(out=dst, in_=ps[:])
                        else:
                            nc.scalar.copy(out=dst, in_=ps[:])
                    reg = table_flat[base * W:(base + GA * P) * W].rearrange(
                        "(p f) -> p f", p=P)
                    if gi % 2 == 0:
                        nc.gpsimd.dma_start(out=reg, in_=gt[:])
                    else:
                        nc.sync.dma_start(out=reg, in_=gt[:])

            # ---------------- message-pass layer helper ------------------
            def run_layer(tab, off_g, off_p, wid, out_cb, gpool, rpool,
                          zpool, ident16, zg_open=False):
                for s in range(NB):
                    abl = os.environ.get("KERNEL_ABLATE", "")
                    gbs = []
                    for w in range(cfg.NW):
                        kw = int(Kw[s, w])
                        if kw == 0:
                            gbs.append(None)
                            continue
                        gb = gpool.tile([P, kw, W], dt16, tag=f"gb{w}")
                        # hw dma_gather breaks above 1024 indices per call
                        for k0 in range(0, kw, 8):
                            kk = min(8, kw - k0)
                            nc.gpsimd.dma_gather(
                                out_ap=gb[:, k0:k0 + kk, :],
                                in_ap=tab[w * cfg.WROWS:(w + 1) * cfg.WROWS, :],
                                idxs_ap=idx_sb[:, (C0[s, w] + k0) * 8:
                                               (C0[s, w] + k0 + kk) * 8],
                                num_idxs=kk * P, num_idxs_reg=kk * P,
                                elem_size=W)
                        gbs.append(gb)
                    m = int(Kw[s].sum())
                    rw = rpool.tile([P, m * wid], dt16, tag="rw")
                    zg = zpool.tile([P, wid], dt32, tag="zg")
                    zp = zpool.tile([P, wid], dt32, tag="zp")
                    j = 0
                    for w in range(cfg.NW):
                        kw = int(Kw[s, w])
                        for k in range(kw):
                            nc.vector.tensor_scalar(
                                out=rw[:, j * wid:(j + 1) * wid],
                                in0=gbs[w][:, k, off_p:off_p + wid],
                                scalar1=sval_sb[:, C0[s, w] + k:
                                                C0[s, w] + k + 1],
                                scalar2=None, op0=AOT.mult)
                            j += 1
                    j = 0
                    for w in range(cfg.NW):
                        kw = int(Kw[s, w])
                        for k in range(kw):
                            nc.tensor.matmul(
                                out=zg[:], lhsT=ident16[:],
                                rhs=gbs[w][:, k, off_g:off_g + wid],
                                start=(j == 0),
                                stop=(j == m - 1) and not zg_open)
                            j += 1
                    j = 0
                    for w in range(cfg.NW):
                        kw = int(Kw[s, w])
                        for k in range(kw):
                            nc.tensor.matmul(
                                out=zp[:], lhsT=ident16[:],
                                rhs=rw[:, j * wid:(j + 1) * wid],
                                start=(j == 0), stop=(j == m - 1))
                            j += 1
                    out_cb(s, zg, zp)

            from concourse.masks import make_identity
            ident16 = cpool.tile([P, P], dt16)
            make_identity(nc, ident16[:])

            # ---------------- L1 -----------------------------------------
            with tc.tile_pool(name="l1_g", bufs=3) as gpool, \
                 tc.tile_pool(name="l1_r", bufs=3) as rpool, \
                 tc.tile_pool(name="l1_z", bufs=3, space="PSUM") as zpool, \
                 tc.tile_pool(name="l1_h", bufs=3) as hpool, \
                 tc.tile_pool(name="l1_t", bufs=3) as tpool, \
                 tc.tile_pool(name="l1_y", bufs=2, space="PSUM") as ypool, \
                 tc.tile_pool(name="l1_o", bufs=2) as opool:

                state = {"yt": None}

                def l1_cb(s, zg, zp):
                    t = s % GL
                    if t == 0:
                        yt_new = opool.tile([P, GL, W], dt16, tag="yt")
                        state["yt"] = yt_new
                    yt = state["yt"]
                    if has_b1:
                        nc.vector.tensor_tensor(out=zg[:], in0=zg[:],
                                                in1=b1sb[:], op=AOT.add)
                    hg = hpool.tile([P, H], dt16, tag="hg")
                    nc.vector.tensor_scalar(out=hg[:], in0=zg[:],
                                            scalar1=disg_sb[:, s:s + 1],
                                            scalar2=0.0, op0=AOT.mult,
                                            op1=AOT.max)
                    hp = hpool.tile([P, H], dt16, tag="hp")
                    if has_b1:
                        zpb = hpool.tile([P, H], dt32, tag="zpb")
                        nc.vector.tensor_tensor(out=zpb[:], in0=zp[:],
                                                in1=b1sb[:], op=AOT.add)
                        nc.vector.tensor_scalar(out=hp[:], in0=zpb[:],
                                                scalar1=0.0, scalar2=None,
                                                op0=AOT.max)
                    else:
                        nc.vector.tensor_scalar(out=hp[:], in0=zp[:],
                                                scalar1=0.0, scalar2=None,
                                                op0=AOT.max)
                    abl = os.environ.get("KERNEL_ABLATE", "")
                    hgT = tpool.tile([P, P], dt16, tag="hgT")
                    hpT = tpool.tile([P, P], dt16, tag="hpT")
                    if "notrans" in abl:
                        nc.vector.tensor_copy(out=hgT[:], in_=hg[:])
                        nc.vector.tensor_copy(out=hpT[:], in_=hp[:])
                    elif "sptrans" in abl:
                        nc.sync.dma_start_transpose(out=hgT[:], in_=hg[:])
                        nc.sync.dma_start_transpose(out=hpT[:], in_=hp[:])
                    else:
                        nc.sync.dma_start_transpose(out=hgT[:], in_=hg[:])
                        nc.scalar.dma_start_transpose(out=hpT[:], in_=hp[:])
                    yps = ypool.tile([P, W], dt32, tag="yps")
                    nc.tensor.matmul(out=yps[:, 0:O], lhsT=hgT[:],
                                     rhs=w2sb[:], start=True, stop=True)
                    nc.tensor.matmul(out=yps[:, O:W], lhsT=hpT[:],
                                     rhs=w2sb[:], start=True, stop=True)
                    nc.scalar.activation(out=yt[:, t, :], in_=yps[:],
                                         func=AFT.Copy,
                                         scale=disg_sb[:, s:s + 1])
                    if t == GL - 1:
                        g = s // GL
                        base = g * GL * P * W
                        reg = yshard_flat[base:base + GL * P * W].rearrange(
                            "(p f) -> p f", p=P)
                        nc.sync.dma_start(out=reg, in_=yt[:].rearrange(
                            "p t w -> p (t w)"))

                phases = os.environ.get("KERNEL_PHASES", "full")
                if phases != "a":
                    run_layer(table.ap(), 0, 0, W, l1_cb, gpool, rpool,
                              zpool, ident16, zg_open=has_b1)
            if phases in ("l1", "a"):
                with tc.tile_pool(name="fin", bufs=1) as fpool:
                    zt = fpool.tile([P, O], dt32)
                    nc.vector.memset(zt[:], 0.0)
                    for g in range(cfg.NGRP * GL):
                        nc.sync.dma_start(
                            out=outp.ap().rearrange(
                                "(a p) o -> p a o", p=P)[:, g, :],
                            in_=zt[:])

            if phases == "full":
                nc.gpsimd.collective_compute(
                    "AllGather", AOT.bypass, replica_groups=groups_all,
                    ins=[y_shard.ap().opt()], outs=[y_full.ap().opt()])

            # ---------------- L2 -----------------------------------------
            if phases != "full":
                pass
            else:
              with tc.tile_pool(name="l2_g", bufs=3) as gpool, \
                   tc.tile_pool(name="l2_r", bufs=3) as rpool, \
                   tc.tile_pool(name="l2_z", bufs=3, space="PSUM") as zpool, \
                   tc.tile_pool(name="l2_e", bufs=4) as epool, \
                   tc.tile_pool(name="l2_o", bufs=2) as opool:

                  state2 = {"ot": None}

                  def l2_cb(s, zg, zp):
                      t = s % GL
                      if t == 0:
                          ot_new = opool.tile([P, GL, O], dt32, tag="ot")
                          state2["ot"] = ot_new
                      ot = state2["ot"]
                      zgs = epool.tile([P, O], dt32, tag="zgs")
                      nc.scalar.activation(out=zgs[:], in_=zg[:], func=AFT.Copy,
                                           scale=disg_sb[:, s:s + 1])
                      zps = epool.tile([P, O], dt32, tag="zps")
                      nc.vector.tensor_copy(out=zps[:], in_=zp[:])
                      pg = epool.tile([P, O], dt32, tag="pg")
                      nc.vector.tensor_tensor(out=pg[:], in0=zgs[:],
                                              in1=dw_sb[:], op=AOT.mult)
                      lg = epool.tile([P, 1], dt32, tag="lg")
                      nc.vector.tensor_reduce(out=lg[:], in_=pg[:],
                                              axis=mybir.AxisListType.X,
                                              op=AOT.add)
                      pp2 = epool.tile([P, O], dt32, tag="pp2")
                      nc.vector.tensor_tensor(out=pp2[:], in0=zps[:],
                                              in1=dw_sb[:], op=AOT.mult)
                      lp = epool.tile([P, 1], dt32, tag="lp")
                      nc.vector.tensor_reduce(out=lp[:], in_=pp2[:],
                                              axis=mybir.AxisListType.X,
                                              op=AOT.add)
                      dl = epool.tile([P, 1], dt32, tag="dl")
                      nc.vector.tensor_tensor(out=dl[:], in0=lg[:], in1=lp[:],
                                              op=AOT.subtract)
                      wg = epool.tile([P, 1], dt32, tag="wg")
                      nc.scalar.activation(out=wg[:], in_=dl[:],
                                           func=AFT.Sigmoid)
                      tdiff = epool.tile([P, O], dt32, tag="tdiff")
                      nc.vector.tensor_tensor(out=tdiff[:], in0=zgs[:],
                                              in1=zps[:], op=AOT.subtract)
                      if has_b2:
                          bl = epool.tile([P, O], dt32, tag="bl")
                          nc.vector.scalar_tensor_tensor(
                              out=bl[:], in0=tdiff[:], scalar=wg[:],
                              in1=zps[:], op0=AOT.mult, op1=AOT.add)
                          nc.vector.tensor_tensor(out=ot[:, t, :], in0=bl[:],
                                                  in1=b2sb[:], op=AOT.add)
                      else:
                          nc.vector.scalar_tensor_tensor(
                              out=ot[:, t, :], in0=tdiff[:], scalar=wg[:],
                              in1=zps[:], op0=AOT.mult, op1=AOT.add)
                      if t == GL - 1:
                          g = s // GL
                          base = g * GL * P * O
                          reg = out_flat[base:base + GL * P * O].rearrange(
                              "(p f) -> p f", p=P)
                          nc.scalar.dma_start(
                              out=reg, in_=ot[:].rearrange("p t o -> p (t o)"))

                  run_layer(y_full.ap(), 0, O, O, l2_cb, gpool, rpool, zpool,
                            ident16)

    nc.compile()
    return nc


_BUILD_CACHE = {}


def _get_program(cfg, key):
    k = (cfg.N, cfg.E) + (key if isinstance(key, tuple) else tuple(key),)
    if k not in _BUILD_CACHE:
        _BUILD_CACHE[k] = build_program(cfg, key)
    return _BUILD_CACHE[k]


LAST_RESULTS = None


def _run(cfg, inputs):
    from concourse.bass_utils import run_bass_kernel_spmd
    global LAST_RESULTS
    in_maps, key, meta = _preprocess(cfg, **inputs)
    nc = _get_program(cfg, key)
    trace = bool(int(os.environ.get("KERNEL_TRACE", "0")))
    res = run_bass_kernel_spmd(nc, in_maps, core_ids=list(range(cfg.ncores)),
                               trace=trace)
    LAST_RESULTS = res
    full = np.concatenate([res.results[c]["out"] for c in range(cfg.ncores)],
                          axis=0)
    trow_of_rank = meta["trow_of_rank"]
    order_ext = meta["order_ext"]
    out = np.empty((cfg.N, cfg.O), dtype=np.float32)
    real = order_ext < cfg.N
    out[order_ext[real]] = full[trow_of_rank[real]]
    return np.ascontiguousarray(out)


def kernel(x, edge_index, ppmi_edge_weight, W1, b1, W2, b2, dense_w, dense_b):
    return _run(FULL, dict(x=x, edge_index=edge_index,
                           ppmi_edge_weight=ppmi_edge_weight, W1=W1, b1=b1,
                           W2=W2, b2=b2, dense_w=dense_w, dense_b=dense_b))

